# revision 12
# baseline (speedup 1.0000x reference)
"""Trainium2 Bass kernel for nn_MultiHeadAttention_78864189489198.

Symmetric-scores fp8 variant.

S = Q Q^T is symmetric, so exp(S) is too: compute scores/exp only for
blocks (v, u) with u >= v (136/256 of the work), and realize each
lower-triangle contribution directly from the stored upper block:

  row use   (YT):  YT[:, u]  += Qnat_v^T @ E[v, u...]     (fp16 matmul)
  mirror use (Y):  Y[u, :]   += E[v, u]^T @ [Qnat_v | 1]  (fp16 matmul)
                   -> transposed on PE into YT layout, ones column
                      accumulates the missing row-sum part of r.

This halves the ACT exp stream (the v2 pacer) at the cost of ~15% more
PE matmul columns and a PE transpose per mirror block.

Everything else as v2.1: fp8 hi/lo DoubleRow Qproj, fp8 DR scores from
qt8 [64,2,...], fp16 PV/outproj, fp16 partial outputs summed on host.
"""

import os

import numpy as np
import ml_dtypes

import concourse.bass as bass
import concourse.mybir as mybir
import concourse.tile as tile
from concourse import bacc
from concourse.bass_utils import run_bass_kernel_spmd
from concourse.masks import make_identity
from contextlib import ExitStack

P = 128
N = 2048
D = 2048
KP = 8
HG = 4
HD = 128
HCOLS = HG * HD
SP = N // 512
NCH = N // P
SCALE = HD ** -0.5
C_BIAS = 9.0
WQS = 64.0
Q8S = 1.0 / 16.0

f32 = mybir.dt.float32
f32r = mybir.dt.float32r
f16 = mybir.dt.float16
e4 = mybir.dt.float8e4
DR = mybir.MatmulPerfMode.DoubleRow

_CACHE = {}


def build_nc():
    nc = bacc.Bacc("TRN2", target_bir_lowering=False, debug=False)
    xh = nc.dram_tensor("xh", [P, SP, KP, 2, 512], e4, kind="ExternalInput")
    xl = nc.dram_tensor("xl", [P, SP, KP, 2, 512], e4, kind="ExternalInput")
    wqh = nc.dram_tensor("wqh", [P, KP, 2, HCOLS], e4, kind="ExternalInput")
    wql = nc.dram_tensor("wql", [P, KP, 2, HCOLS], e4, kind="ExternalInput")
    wo8h = nc.dram_tensor("wo8h", [P, HG, D], e4, kind="ExternalInput")
    wo8l = nc.dram_tensor("wo8l", [P, HG, D], e4, kind="ExternalInput")
    out = nc.dram_tensor("out", [N, D], f16, kind="ExternalOutput")
    out3 = out.rearrange("(a p) n -> p a n", p=P)

    with (
        nc.allow_low_precision(reason="fp8/fp16 dataflow is intentional"),
        tile.TileContext(nc) as tc,
        ExitStack() as ctx,
    ):
        const_pool = ctx.enter_context(tc.tile_pool(name="const", bufs=1))
        qt8_pool = ctx.enter_context(tc.tile_pool(name="qt8", bufs=1))
        qn_pool = ctx.enter_context(tc.tile_pool(name="qn", bufs=1))
        es_pool = ctx.enter_context(tc.tile_pool(name="es", bufs=19))
        rr_pool = ctx.enter_context(tc.tile_pool(name="rr", bufs=2))
        rbc_pool = ctx.enter_context(tc.tile_pool(name="rbc", bufs=2))
        yt_pool = ctx.enter_context(tc.tile_pool(name="yt", bufs=2))
        wo_pool = ctx.enter_context(tc.tile_pool(name="wo", bufs=1))
        y8_pool = ctx.enter_context(tc.tile_pool(name="y8", bufs=1))
        o_pool = ctx.enter_context(tc.tile_pool(name="osb", bufs=3))
        ynm_pool = ctx.enter_context(tc.tile_pool(name="ynm", bufs=3))
        ps_s = ctx.enter_context(tc.tile_pool(name="ps_s", bufs=2, space="PSUM"))
        ps_b = ctx.enter_context(tc.tile_pool(name="ps_b", bufs=4, space="PSUM"))

        idr = const_pool.tile([P, P], f32r, tag="idr")
        id16 = const_pool.tile([P, P], f16, tag="id16")
        cbias = const_pool.tile([P, 1], f32, tag="cbias")
        nc.gpsimd.memset(cbias[:], -C_BIAS)

        qt8 = qt8_pool.tile([64, 2, HG, N], e4, tag="qt8")
        # qn[:, a, c, 0:128] = Qnat chunk a of head c; col 128 = ones
        qn_sb = qn_pool.tile([P, NCH, HG, 129], f16, tag="qn")
        nc.gpsimd.memset(qn_sb[:, :, :, 128:129], 1.0)
        yts = {}
        rrechs = {}
        rmirs = {}
        ess = {}

        def new_head_state(c):
            rrechs[c] = rr_pool.tile([P, NCH, 2], f32, tag="rrech", name=f"rr{c}")
            rmirs[c] = rr_pool.tile([P, NCH], f32, tag="rmir", name=f"rm{c}")
            nc.vector.memset(rrechs[c][:], 0.0)
            nc.vector.memset(rmirs[c][:], 0.0)
            yts[c] = yt_pool.tile([P, N], f16, tag="yt", name=f"yt{c}")
            nc.gpsimd.memset(yts[c][:, 0:P], 0.0)
            ess[c] = []

        def scores_piece(c, v, h, es):
            # cols [max(128v, 1024h), 1024(h+1))
            lo = max(P * v, 1024 * h)
            hi = 1024 * (h + 1)
            if lo >= hi:
                return
            ps = ps_s.tile([P, 1024], f32, tag="s")
            col = lo
            while col < hi:
                w = min(512 - col % 512, hi - col)
                nc.tensor.matmul(
                    ps[:, col - 1024 * h:col - 1024 * h + w],
                    qt8[:, :, c, v * P:(v + 1) * P],
                    qt8[:, :, c, col:col + w],
                    start=True,
                    stop=True,
                    perf_mode=DR,
                )
                col += w
            nc.scalar.activation(
                es[:, lo:hi],
                ps[:, lo - 1024 * h:1024],
                mybir.ActivationFunctionType.Exp,
                bias=cbias[:, 0:1],
                scale=SCALE / 16.0,
                accum_out=rrechs[c][:, v, h:h + 1],
            )

        def scores_row(c, v):
            es = es_pool.tile([P, N], f16, tag="es")
            scores_piece(c, v, 0, es)
            scores_piece(c, v, 1, es)
            ess[c].append(es)

        def mirror_unit(c, u):
            # Y[u-chunk, :] += sum_{v<u} E[v, u-block]^T @ [Qnat_v | 1]
            pm = ps_b.tile([P, 132], f32, tag="b", name=f"pm{c}_{u}")
            for v in range(u):
                nc.tensor.matmul(
                    pm[:, 0:129],
                    ess[c][v][:, u * P:(u + 1) * P],
                    qn_sb[:, v, c, :],
                    start=(v == 0),
                    stop=(v == u - 1),
                )
            ynm = ynm_pool.tile([P, P], f16, tag="ynm")
            nc.vector.tensor_copy(ynm[:], pm[:, 0:P])
            nc.vector.tensor_copy(rmirs[c][:, u:u + 1], pm[:, 128:129])
            ptm = ps_b.tile([P, P], f16, tag="b", name=f"ptm{c}_{u}")
            nc.tensor.transpose(ptm[:], ynm[:], id16[:])
            nc.vector.tensor_copy(yts[c][:, u * P:(u + 1) * P], ptm[:])

        def r_chain(c):
            rrec = rr_pool.tile([P, NCH], f32, tag="rrec")
            nc.vector.tensor_reduce(
                rrec[:], rrechs[c][:], mybir.AxisListType.X, mybir.AluOpType.add
            )
            nc.vector.tensor_tensor(
                rrec[:], rrec[:], rmirs[c][:], mybir.AluOpType.add
            )
            rrec2 = rr_pool.tile([P, NCH], f32r, tag="rrec2")
            nc.vector.reciprocal(rrec2[:], rrec[:])
            prt = ps_b.tile([NCH, P], f32r, tag="b")
            nc.tensor.transpose(prt[:], rrec2[:], idr[:])
            rt16 = rr_pool.tile([NCH, P], f16, tag="rt16")
            nc.vector.tensor_scalar_mul(rt16[:], prt[:], 16.0)
            rbc = rbc_pool.tile([P, N], f16, tag="rbc")
            nc.sync.dma_start(rbc[0:1, :], rt16[:, :])
            nc.gpsimd.partition_broadcast(rbc[:], rbc[0:1, :])
            return rbc

        with (
            tc.tile_pool(name="xt", bufs=4) as xt_pool,
            tc.tile_pool(name="wq", bufs=1) as wq_pool,
            tc.tile_pool(name="q16r", bufs=3) as q16r_pool,
            tc.tile_pool(name="q8f", bufs=4) as q8f_pool,
        ):
            make_identity(nc, id16[:])
            nc.vector.tensor_copy(idr[:], id16[:])

            wqh_sb = wq_pool.tile([P, KP, 2, HCOLS], e4, tag="wqh")
            wql_sb = wq_pool.tile([P, KP, 2, HCOLS], e4, tag="wql")
            xsp = {}
            for t in range(SP):
                xsp[t] = (
                    xt_pool.tile([P, KP, 2, 512], e4, tag="xt", name=f"xh{t}"),
                    xt_pool.tile([P, KP, 2, 512], e4, tag="xt", name=f"xl{t}"),
                )
            nc.sync.dma_start(wqh_sb[:, 0:4], wqh[:, 0:4])
            nc.scalar.dma_start(xsp[0][0][:], xh[:, 0])
            nc.sync.dma_start(wqh_sb[:, 4:8], wqh[:, 4:8])
            nc.scalar.dma_start(wql_sb[:], wql[:])
            nc.gpsimd.dma_start(xsp[0][1][:], xl[:, 0])
            nc.scalar.dma_start(xsp[1][1][:], xl[:, 1])
            nc.gpsimd.dma_start(xsp[1][0][:], xh[:, 1])
            nc.sync.dma_start(xsp[2][0][:], xh[:, 2])
            nc.scalar.dma_start(xsp[3][1][:], xl[:, 3])
            nc.gpsimd.dma_start(xsp[2][1][:], xl[:, 2])
            nc.sync.dma_start(xsp[3][0][:], xh[:, 3])
            edum = rr_pool.tile([P, 1], f32, tag="edum")
            nc.scalar.activation(
                edum[:], cbias[:, 0:1], mybir.ActivationFunctionType.Exp
            )

            q8flats = {}
            pend_tr = []

            def qproj_unit(c, t):
                ps = ps_b.tile([P, 512], f32, tag="b")
                th, tl = xsp[t]
                combos = [(wqh_sb, th, kp) for kp in range(KP)]
                combos += [(wql_sb, th, kp) for kp in range(KP)]
                combos += [(wqh_sb, tl, kp) for kp in range(KP)]
                for i, (w, x, kp) in enumerate(combos):
                    nc.tensor.matmul(
                        ps[:],
                        w[:, kp, :, c * P:(c + 1) * P],
                        x[:, kp],
                        start=(i == 0),
                        stop=(i == len(combos) - 1),
                        perf_mode=DR,
                    )
                q16 = q16r_pool.tile([P, 512], f16, tag="q16")
                nc.vector.tensor_copy(q16[:], ps[:])
                if c not in q8flats:
                    q8flats[c] = q8f_pool.tile([P, N], e4, tag="q8f", name=f"q8f{c}")
                nc.gpsimd.tensor_scalar_mul(
                    q8flats[c][:, t * 512:(t + 1) * 512], q16[:], 1.0 / 16.0
                )
                if pend_tr:
                    pend_tr.pop(0)()

                def transposes(c=c, t=t, q16=q16):
                    for j in range(4):
                        pt = ps_b.tile([P, P], f16, tag="b")
                        nc.tensor.transpose(
                            pt[:], q16[:, j * P:(j + 1) * P], id16[:]
                        )
                        nc.vector.tensor_scalar_mul(
                            qn_sb[:, t * 4 + j, c, 0:P], pt[:], 1.0 / 1024.0
                        )

                pend_tr.append(transposes)

            def fold(c, lo_t, hi_t):
                nc.sync.dma_start(
                    qt8[:, :, c, lo_t * 512:hi_t * 512],
                    q8flats[c][:, lo_t * 512:hi_t * 512],
                )

            for t in range(2):
                for c in range(HG):
                    qproj_unit(c, t)
            fold(0, 0, 2)

            while pend_tr:
                pend_tr.pop(0)()
            new_head_state(0)
            # phase 1: h=0 pieces for rows v<8 (need spans 0-1 only)
            for v in range(8):
                es = es_pool.tile([P, N], f16, tag="es")
                scores_piece(0, v, 0, es)
                ess[0].append(es)

            for c in range(HG):
                qproj_unit(c, 2)
            qproj_unit(0, 3)
            fold(0, 2, 4)
            fold(1, 0, 3)

            tailq = []
            for c in range(1, HG):
                tailq.append(lambda c=c: qproj_unit(c, 3))
                if c == 1:
                    tailq.append(lambda: fold(1, 3, 4))
                else:
                    tailq.append(lambda c=c: fold(c, 0, 4))

            # phase 2: finish head-0 rows; mirrors trail 2 rows behind so
            # their psum slots rotate through freed qproj/transpose slots
            for v in range(NCH):
                if v < 8:
                    scores_piece(0, v, 1, ess[0][v])
                else:
                    scores_row(0, v)
                if v >= 2:
                    mirror_unit(0, v - 1)
                npop = (len(tailq) + NCH - 1 - v) // (NCH - v)
                for _ in range(npop):
                    if tailq:
                        tailq.pop(0)()
            while tailq:
                tailq.pop(0)()
            while pend_tr:
                pend_tr.pop(0)()
            mirror_unit(0, NCH - 1)

        wo8h_sb = wo_pool.tile([P, HG, D], e4, tag="wo8h")
        wo8l_sb = wo_pool.tile([P, HG, D], e4, tag="wo8l")
        nc.sync.dma_start(wo8h_sb[:], wo8h[:])
        nc.gpsimd.dma_start(wo8l_sb[:], wo8l[:])
        y8h = y8_pool.tile([P, HG, N], e4, tag="y8h")
        y8l = y8_pool.tile([P, HG, N], e4, tag="y8l")

        def outproj_unit(a):
            ot = o_pool.tile([P, D], f16, tag="ot")
            for d4 in range(SP):
                ps = ps_b.tile([P, 512], f32, tag="b")
                combos = []
                for cc in (0, 2):
                    combos += [
                        (y8h, wo8h_sb, cc),
                        (y8h, wo8l_sb, cc),
                        (y8l, wo8h_sb, cc),
                    ]
                for i, (yy, ww, cc) in enumerate(combos):
                    nc.tensor.matmul(
                        ps[:],
                        yy[:, cc:cc + 2, a * P:(a + 1) * P],
                        ww[:, cc:cc + 2, d4 * 512:(d4 + 1) * 512],
                        start=(i == 0),
                        stop=(i == len(combos) - 1),
                        perf_mode=DR,
                    )
                if d4 % 2 == 0:
                    nc.vector.tensor_scalar_mul(
                        ot[:, d4 * 512:(d4 + 1) * 512], ps[:], 1.0 / 64.0
                    )
                else:
                    nc.scalar.mul(ot[:, d4 * 512:(d4 + 1) * 512], ps[:], 1.0 / 64.0)
            eng = nc.sync if a % 2 == 0 else nc.gpsimd
            eng.dma_start(out3[:, a, :], ot[:])

        # ================= heads pipeline =================
        # head c iteration: scores+mirror of head c, PV of head pc=c-1
        # with early per-quarter evac + norm.
        for c in range(1, HG + 1):
            pc = c - 1
            rbc = r_chain(pc)
            if c < HG:
                new_head_state(c)
            psy = {}
            for s in range(SP):
                psy[s] = ps_b.tile([P, 512], f32, tag="b", name=f"psy{c}_{s}")
            mirrorq = []
            pes = ess[pc]

            def pv_row(v):
                for s in range(v // 4, SP):
                    lo = max(s * 512, v * P)
                    hi = (s + 1) * 512
                    nc.tensor.matmul(
                        psy[s][:, lo - s * 512:hi - s * 512],
                        qn_sb[:, v, pc, 0:P],
                        pes[v][:, lo:hi],
                        start=(v == 0),
                        stop=(v == 4 * s + 3),
                    )

            def evac_norm(s):
                yt = yts[pc]
                nc.vector.tensor_tensor(
                    yt[:, s * 512:(s + 1) * 512],
                    psy[s][:],
                    yt[:, s * 512:(s + 1) * 512],
                    mybir.AluOpType.add,
                )
                nc.vector.tensor_tensor(
                    yt[:, s * 512:(s + 1) * 512],
                    yt[:, s * 512:(s + 1) * 512],
                    rbc[:, s * 512:(s + 1) * 512],
                    mybir.AluOpType.mult,
                )
                nc.gpsimd.tensor_copy(
                    y8h[:, pc, s * 512:(s + 1) * 512],
                    yt[:, s * 512:(s + 1) * 512],
                )
                nc.gpsimd.tensor_tensor(
                    y8l[:, pc, s * 512:(s + 1) * 512],
                    yt[:, s * 512:(s + 1) * 512],
                    y8h[:, pc, s * 512:(s + 1) * 512],
                    mybir.AluOpType.subtract,
                )

            for v in range(NCH):
                pv_row(v)
                if c < HG:
                    scores_row(c, v)
                    if v >= 1:
                        mirrorq.append(v)
                if v % 4 == 3:
                    evac_norm(v // 4)
                    while mirrorq:
                        mirror_unit(c, mirrorq.pop(0))
                    if c == HG:
                        for a in range(v - 3, v + 1):
                            outproj_unit(a)

    nc.compile()
    return nc


def _hi_lo(arr):
    hi = arr.astype(ml_dtypes.float8_e4m3)
    lo = (arr - hi.astype(np.float32)).astype(ml_dtypes.float8_e4m3)
    return hi, lo


def _pack_x(xt_hl):
    return np.ascontiguousarray(
        xt_hl.reshape(KP, 2, P, SP, 512).transpose(2, 3, 0, 1, 4)
    )


def kernel(x, Wq, Wo, bo):
    x = np.asarray(x, dtype=np.float32)
    Wq = np.asarray(Wq, dtype=np.float32)
    Wo = np.asarray(Wo, dtype=np.float32)
    bo = np.asarray(bo, dtype=np.float32)
    B = x.shape[0]
    assert B == 2 and x.shape == (B, N, D)
    assert Wq.shape == (D, D) and Wo.shape == (D, D)

    if "nc" not in _CACHE:
        _CACHE["nc"] = build_nc()
    nc = _CACHE["nc"]

    packed_x = []
    for b in range(B):
        hi, lo = _hi_lo(np.ascontiguousarray(x[b].T))
        packed_x.append((_pack_x(hi), _pack_x(lo)))
    in_maps = []
    for core in range(8):
        b, hg = core // 4, core % 4
        wq_s = WQS * Wq[:, hg * HCOLS:(hg + 1) * HCOLS]
        wqh, wql = _hi_lo(np.ascontiguousarray(wq_s))
        wo8h_a, wo8l_a = _hi_lo(
            np.ascontiguousarray(WQS * Wo[hg * HCOLS:(hg + 1) * HCOLS, :])
        )
        in_maps.append(
            {
                "xh": packed_x[b][0],
                "xl": packed_x[b][1],
                "wqh": np.ascontiguousarray(
                    wqh.reshape(KP, 2, P, HCOLS).transpose(2, 0, 1, 3)
                ),
                "wql": np.ascontiguousarray(
                    wql.reshape(KP, 2, P, HCOLS).transpose(2, 0, 1, 3)
                ),
                "wo8h": np.ascontiguousarray(
                    wo8h_a.reshape(HG, P, D).transpose(1, 0, 2)
                ),
                "wo8l": np.ascontiguousarray(
                    wo8l_a.reshape(HG, P, D).transpose(1, 0, 2)
                ),
            }
        )

    res = run_bass_kernel_spmd(nc, in_maps, list(range(8)))
    _CACHE["last_res"] = res
    out = np.zeros((B, N, D), dtype=np.float32)
    for core in range(8):
        b = core // 4
        out[b] += res.results[core]["out"].astype(np.float32)
    out += bo
    return out


# revision 14
# speedup vs baseline: 1.0014x; 1.0014x over previous
"""Trainium2 Bass kernel for nn_MultiHeadAttention_78864189489198.

Symmetric-scores fp8 variant.

S = Q Q^T is symmetric, so exp(S) is too: compute scores/exp only for
blocks (v, u) with u >= v (136/256 of the work), and realize each
lower-triangle contribution directly from the stored upper block:

  row use   (YT):  YT[:, u]  += Qnat_v^T @ E[v, u...]     (fp16 matmul)
  mirror use (Y):  Y[u, :]   += E[v, u]^T @ [Qnat_v | 1]  (fp16 matmul)
                   -> transposed on PE into YT layout, ones column
                      accumulates the missing row-sum part of r.

This halves the ACT exp stream (the v2 pacer) at the cost of ~15% more
PE matmul columns and a PE transpose per mirror block.

Everything else as v2.1: fp8 hi/lo DoubleRow Qproj, fp8 DR scores from
qt8 [64,2,...], fp16 PV/outproj, fp16 partial outputs summed on host.
"""

import os

import numpy as np
import ml_dtypes

import concourse.bass as bass
import concourse.mybir as mybir
import concourse.tile as tile
from concourse import bacc
from concourse.bass_utils import run_bass_kernel_spmd
from concourse.masks import make_identity
from contextlib import ExitStack

P = 128
N = 2048
D = 2048
KP = 8
HG = 4
HD = 128
HCOLS = HG * HD
SP = N // 512
NCH = N // P
SCALE = HD ** -0.5
C_BIAS = 9.0
WQS = 64.0
Q8S = 1.0 / 16.0

f32 = mybir.dt.float32
f32r = mybir.dt.float32r
f16 = mybir.dt.float16
e4 = mybir.dt.float8e4
DR = mybir.MatmulPerfMode.DoubleRow

_CACHE = {}


def build_nc():
    nc = bacc.Bacc("TRN2", target_bir_lowering=False, debug=False)
    xh = nc.dram_tensor("xh", [P, SP, KP, 2, 512], e4, kind="ExternalInput")
    xl = nc.dram_tensor("xl", [P, SP, KP, 2, 512], e4, kind="ExternalInput")
    wqh = nc.dram_tensor("wqh", [P, KP, 2, HCOLS], e4, kind="ExternalInput")
    wql = nc.dram_tensor("wql", [P, KP, 2, HCOLS], e4, kind="ExternalInput")
    wo8h = nc.dram_tensor("wo8h", [P, HG, D], e4, kind="ExternalInput")
    wo8l = nc.dram_tensor("wo8l", [P, HG, D], e4, kind="ExternalInput")
    out = nc.dram_tensor("out", [N, D], f16, kind="ExternalOutput")
    out3 = out.rearrange("(a p) n -> p a n", p=P)

    with (
        nc.allow_low_precision(reason="fp8/fp16 dataflow is intentional"),
        tile.TileContext(nc) as tc,
        ExitStack() as ctx,
    ):
        const_pool = ctx.enter_context(tc.tile_pool(name="const", bufs=1))
        qt8_pool = ctx.enter_context(tc.tile_pool(name="qt8", bufs=1))
        qn_pool = ctx.enter_context(tc.tile_pool(name="qn", bufs=1))
        es_pool = ctx.enter_context(tc.tile_pool(name="es", bufs=19))
        rr_pool = ctx.enter_context(tc.tile_pool(name="rr", bufs=3))
        rbc_pool = ctx.enter_context(tc.tile_pool(name="rbc", bufs=2))
        yt_pool = ctx.enter_context(tc.tile_pool(name="yt", bufs=2))
        wo_pool = ctx.enter_context(tc.tile_pool(name="wo", bufs=1))
        y8_pool = ctx.enter_context(tc.tile_pool(name="y8", bufs=1))
        o_pool = ctx.enter_context(tc.tile_pool(name="osb", bufs=3))
        ynm_pool = ctx.enter_context(tc.tile_pool(name="ynm", bufs=3))
        ps_s = ctx.enter_context(tc.tile_pool(name="ps_s", bufs=2, space="PSUM"))
        ps_b = ctx.enter_context(tc.tile_pool(name="ps_b", bufs=4, space="PSUM"))

        idr = const_pool.tile([P, P], f32r, tag="idr")
        id16 = const_pool.tile([P, P], f16, tag="id16")
        cbias = const_pool.tile([P, 1], f32, tag="cbias")
        nc.gpsimd.memset(cbias[:], -C_BIAS)

        qt8 = qt8_pool.tile([64, 2, HG, N], e4, tag="qt8")
        # qn[:, a, c, 0:128] = Qnat chunk a of head c; col 128 = ones
        qn_sb = qn_pool.tile([P, NCH, HG, 129], f16, tag="qn")
        nc.gpsimd.memset(qn_sb[:, :, :, 128:129], 1.0)
        yts = {}
        rrechs = {}
        rmirs = {}
        ess = {}

        def new_head_state(c):
            rrechs[c] = rr_pool.tile([P, NCH, 2], f32, tag="rrech", name=f"rr{c}")
            rmirs[c] = rr_pool.tile([P, NCH], f32, tag="rmir", name=f"rm{c}")
            nc.vector.memset(rrechs[c][:], 0.0)
            nc.vector.memset(rmirs[c][:], 0.0)
            yts[c] = yt_pool.tile([P, N], f16, tag="yt", name=f"yt{c}")
            nc.gpsimd.memset(yts[c][:, 0:P], 0.0)
            ess[c] = []

        def scores_piece(c, v, h, es):
            # cols [max(128v, 1024h), 1024(h+1))
            lo = max(P * v, 1024 * h)
            hi = 1024 * (h + 1)
            if lo >= hi:
                return
            ps = ps_s.tile([P, 1024], f32, tag="s")
            col = lo
            while col < hi:
                w = min(512 - col % 512, hi - col)
                nc.tensor.matmul(
                    ps[:, col - 1024 * h:col - 1024 * h + w],
                    qt8[:, :, c, v * P:(v + 1) * P],
                    qt8[:, :, c, col:col + w],
                    start=True,
                    stop=True,
                    perf_mode=DR,
                )
                col += w
            nc.scalar.activation(
                es[:, lo:hi],
                ps[:, lo - 1024 * h:1024],
                mybir.ActivationFunctionType.Exp,
                bias=cbias[:, 0:1],
                scale=SCALE / 16.0,
                accum_out=rrechs[c][:, v, h:h + 1],
            )

        def scores_row(c, v):
            es = es_pool.tile([P, N], f16, tag="es")
            scores_piece(c, v, 0, es)
            scores_piece(c, v, 1, es)
            ess[c].append(es)

        def mirror_unit(c, u):
            # Y[u-chunk, :] += sum_{v<u} E[v, u-block]^T @ [Qnat_v | 1]
            pm = ps_b.tile([P, 132], f32, tag="b", name=f"pm{c}_{u}")
            for v in range(u):
                nc.tensor.matmul(
                    pm[:, 0:129],
                    ess[c][v][:, u * P:(u + 1) * P],
                    qn_sb[:, v, c, :],
                    start=(v == 0),
                    stop=(v == u - 1),
                )
            ynm = ynm_pool.tile([P, P], f16, tag="ynm")
            nc.vector.tensor_copy(ynm[:], pm[:, 0:P])
            nc.vector.tensor_copy(rmirs[c][:, u:u + 1], pm[:, 128:129])
            ptm = ps_b.tile([P, P], f16, tag="b", name=f"ptm{c}_{u}")
            nc.tensor.transpose(ptm[:], ynm[:], id16[:])
            nc.vector.tensor_copy(yts[c][:, u * P:(u + 1) * P], ptm[:])

        def r_chain(c):
            rrec = rr_pool.tile([P, NCH], f32, tag="rrec")
            nc.vector.tensor_reduce(
                rrec[:], rrechs[c][:], mybir.AxisListType.X, mybir.AluOpType.add
            )
            nc.vector.tensor_tensor(
                rrec[:], rrec[:], rmirs[c][:], mybir.AluOpType.add
            )
            rrec2 = rr_pool.tile([P, NCH], f32r, tag="rrec2")
            nc.vector.reciprocal(rrec2[:], rrec[:])
            prt = ps_b.tile([NCH, P], f32r, tag="b")
            nc.tensor.transpose(prt[:], rrec2[:], idr[:])
            rt16 = rr_pool.tile([NCH, P], f16, tag="rt16")
            nc.vector.tensor_scalar_mul(rt16[:], prt[:], 16.0)
            rbc = rbc_pool.tile([P, N], f16, tag="rbc")
            nc.sync.dma_start(rbc[0:1, :], rt16[:, :])
            nc.gpsimd.partition_broadcast(rbc[:], rbc[0:1, :])
            return rbc

        with (
            tc.tile_pool(name="xt", bufs=5) as xt_pool,
            tc.tile_pool(name="wq", bufs=1) as wq_pool,
            tc.tile_pool(name="q16r", bufs=3) as q16r_pool,
            tc.tile_pool(name="q8f", bufs=4) as q8f_pool,
        ):
            make_identity(nc, id16[:])
            nc.vector.tensor_copy(idr[:], id16[:])

            wqh_sb = wq_pool.tile([P, KP, 2, HCOLS], e4, tag="wqh")
            wql_sb = wq_pool.tile([P, KP, 2, HCOLS], e4, tag="wql")
            xsp = {}
            for t in range(SP):
                xsp[t] = (
                    xt_pool.tile([P, KP, 2, 512], e4, tag="xt", name=f"xh{t}"),
                    xt_pool.tile([P, KP, 2, 512], e4, tag="xt", name=f"xl{t}"),
                )
            nc.sync.dma_start(wqh_sb[:, 0:4], wqh[:, 0:4])
            nc.scalar.dma_start(xsp[0][0][:], xh[:, 0])
            nc.sync.dma_start(wqh_sb[:, 4:8], wqh[:, 4:8])
            nc.scalar.dma_start(wql_sb[:], wql[:])
            nc.gpsimd.dma_start(xsp[0][1][:], xl[:, 0])
            nc.scalar.dma_start(xsp[1][1][:], xl[:, 1])
            nc.gpsimd.dma_start(xsp[1][0][:], xh[:, 1])
            nc.sync.dma_start(xsp[2][0][:], xh[:, 2])
            nc.scalar.dma_start(xsp[3][1][:], xl[:, 3])
            nc.gpsimd.dma_start(xsp[2][1][:], xl[:, 2])
            nc.sync.dma_start(xsp[3][0][:], xh[:, 3])
            edum = rr_pool.tile([P, 1], f32, tag="edum")
            nc.scalar.activation(
                edum[:], cbias[:, 0:1], mybir.ActivationFunctionType.Exp
            )

            q8flats = {}
            pend_tr = []

            def qproj_unit(c, t):
                ps = ps_b.tile([P, 512], f32, tag="b")
                th, tl = xsp[t]
                combos = [(wqh_sb, th, kp) for kp in range(KP)]
                combos += [(wql_sb, th, kp) for kp in range(KP)]
                combos += [(wqh_sb, tl, kp) for kp in range(KP)]
                for i, (w, x, kp) in enumerate(combos):
                    nc.tensor.matmul(
                        ps[:],
                        w[:, kp, :, c * P:(c + 1) * P],
                        x[:, kp],
                        start=(i == 0),
                        stop=(i == len(combos) - 1),
                        perf_mode=DR,
                    )
                q16 = q16r_pool.tile([P, 512], f16, tag="q16")
                nc.vector.tensor_copy(q16[:], ps[:])
                if c not in q8flats:
                    q8flats[c] = q8f_pool.tile([P, N], e4, tag="q8f", name=f"q8f{c}")
                nc.gpsimd.tensor_scalar_mul(
                    q8flats[c][:, t * 512:(t + 1) * 512], q16[:], 1.0 / 16.0
                )
                if pend_tr:
                    pend_tr.pop(0)()

                def transposes(c=c, t=t, q16=q16):
                    for j in range(4):
                        pt = ps_b.tile([P, P], f16, tag="b")
                        nc.tensor.transpose(
                            pt[:], q16[:, j * P:(j + 1) * P], id16[:]
                        )
                        nc.vector.tensor_scalar_mul(
                            qn_sb[:, t * 4 + j, c, 0:P], pt[:], 1.0 / 1024.0
                        )

                pend_tr.append(transposes)

            def fold(c, lo_t, hi_t):
                nc.sync.dma_start(
                    qt8[:, :, c, lo_t * 512:hi_t * 512],
                    q8flats[c][:, lo_t * 512:hi_t * 512],
                )

            for t in range(2):
                for c in range(HG):
                    qproj_unit(c, t)
            fold(0, 0, 2)

            while pend_tr:
                pend_tr.pop(0)()
            new_head_state(0)
            # phase 1: h=0 pieces for rows v<8 (need spans 0-1 only)
            for v in range(8):
                es = es_pool.tile([P, N], f16, tag="es")
                scores_piece(0, v, 0, es)
                ess[0].append(es)

            for c in range(HG):
                qproj_unit(c, 2)
            qproj_unit(0, 3)
            fold(0, 2, 4)
            fold(1, 0, 3)

            tailq = []
            for c in range(1, HG):
                tailq.append(lambda c=c: qproj_unit(c, 3))
                if c == 1:
                    tailq.append(lambda: fold(1, 3, 4))
                else:
                    tailq.append(lambda c=c: fold(c, 0, 4))

            # phase 2: finish head-0 rows; mirrors trail 2 rows behind so
            # their psum slots rotate through freed qproj/transpose slots
            for v in range(NCH):
                if v < 8:
                    scores_piece(0, v, 1, ess[0][v])
                else:
                    scores_row(0, v)
                if v >= 2:
                    mirror_unit(0, v - 1)
                npop = (len(tailq) + NCH - 1 - v) // (NCH - v)
                for _ in range(npop):
                    if tailq:
                        tailq.pop(0)()
            while tailq:
                tailq.pop(0)()
            while pend_tr:
                pend_tr.pop(0)()
            mirror_unit(0, NCH - 1)

        wo8h_sb = wo_pool.tile([P, HG, D], e4, tag="wo8h")
        wo8l_sb = wo_pool.tile([P, HG, D], e4, tag="wo8l")
        nc.sync.dma_start(wo8h_sb[:], wo8h[:])
        nc.gpsimd.dma_start(wo8l_sb[:], wo8l[:])
        y8h = y8_pool.tile([P, HG, N], e4, tag="y8h")
        y8l = y8_pool.tile([P, HG, N], e4, tag="y8l")

        def outproj_unit(a):
            ot = o_pool.tile([P, D], f16, tag="ot")
            for d4 in range(SP):
                ps = ps_b.tile([P, 512], f32, tag="b")
                combos = []
                for cc in (0, 2):
                    combos += [
                        (y8h, wo8h_sb, cc),
                        (y8h, wo8l_sb, cc),
                        (y8l, wo8h_sb, cc),
                    ]
                for i, (yy, ww, cc) in enumerate(combos):
                    nc.tensor.matmul(
                        ps[:],
                        yy[:, cc:cc + 2, a * P:(a + 1) * P],
                        ww[:, cc:cc + 2, d4 * 512:(d4 + 1) * 512],
                        start=(i == 0),
                        stop=(i == len(combos) - 1),
                        perf_mode=DR,
                    )
                if d4 % 2 == 0:
                    nc.vector.tensor_scalar_mul(
                        ot[:, d4 * 512:(d4 + 1) * 512], ps[:], 1.0 / 64.0
                    )
                else:
                    nc.scalar.mul(ot[:, d4 * 512:(d4 + 1) * 512], ps[:], 1.0 / 64.0)
            eng = nc.sync if a % 2 == 0 else nc.gpsimd
            eng.dma_start(out3[:, a, :], ot[:])

        # ================= heads pipeline =================
        # head c iteration: scores+mirror of head c, PV of head pc=c-1
        # with early per-quarter evac + norm.
        for c in range(1, HG + 1):
            pc = c - 1
            rbc = r_chain(pc)
            if c < HG:
                new_head_state(c)
            psy = {}
            for s in range(SP):
                psy[s] = ps_b.tile([P, 512], f32, tag="b", name=f"psy{c}_{s}")
            mirrorq = []
            pes = ess[pc]

            def pv_row(v):
                for s in range(v // 4, SP):
                    lo = max(s * 512, v * P)
                    hi = (s + 1) * 512
                    nc.tensor.matmul(
                        psy[s][:, lo - s * 512:hi - s * 512],
                        qn_sb[:, v, pc, 0:P],
                        pes[v][:, lo:hi],
                        start=(v == 0),
                        stop=(v == 4 * s + 3),
                    )

            def evac_norm(s):
                yt = yts[pc]
                nc.vector.tensor_tensor(
                    yt[:, s * 512:(s + 1) * 512],
                    psy[s][:],
                    yt[:, s * 512:(s + 1) * 512],
                    mybir.AluOpType.add,
                )
                nc.vector.tensor_tensor(
                    yt[:, s * 512:(s + 1) * 512],
                    yt[:, s * 512:(s + 1) * 512],
                    rbc[:, s * 512:(s + 1) * 512],
                    mybir.AluOpType.mult,
                )
                nc.gpsimd.tensor_copy(
                    y8h[:, pc, s * 512:(s + 1) * 512],
                    yt[:, s * 512:(s + 1) * 512],
                )
                nc.gpsimd.tensor_tensor(
                    y8l[:, pc, s * 512:(s + 1) * 512],
                    yt[:, s * 512:(s + 1) * 512],
                    y8h[:, pc, s * 512:(s + 1) * 512],
                    mybir.AluOpType.subtract,
                )

            for v in range(NCH):
                pv_row(v)
                if c < HG:
                    scores_row(c, v)
                    if v >= 1:
                        mirrorq.append(v)
                if v % 4 == 3:
                    evac_norm(v // 4)
                    while mirrorq:
                        mirror_unit(c, mirrorq.pop(0))
                    if c == HG:
                        for a in range(v - 3, v + 1):
                            outproj_unit(a)

    nc.compile()
    return nc


def _hi_lo(arr):
    hi = arr.astype(ml_dtypes.float8_e4m3)
    lo = (arr - hi.astype(np.float32)).astype(ml_dtypes.float8_e4m3)
    return hi, lo


def _pack_x(xt_hl):
    return np.ascontiguousarray(
        xt_hl.reshape(KP, 2, P, SP, 512).transpose(2, 3, 0, 1, 4)
    )


def kernel(x, Wq, Wo, bo):
    x = np.asarray(x, dtype=np.float32)
    Wq = np.asarray(Wq, dtype=np.float32)
    Wo = np.asarray(Wo, dtype=np.float32)
    bo = np.asarray(bo, dtype=np.float32)
    B = x.shape[0]
    assert B == 2 and x.shape == (B, N, D)
    assert Wq.shape == (D, D) and Wo.shape == (D, D)

    if "nc" not in _CACHE:
        _CACHE["nc"] = build_nc()
    nc = _CACHE["nc"]

    packed_x = []
    for b in range(B):
        hi, lo = _hi_lo(np.ascontiguousarray(x[b].T))
        packed_x.append((_pack_x(hi), _pack_x(lo)))
    in_maps = []
    for core in range(8):
        b, hg = core // 4, core % 4
        wq_s = WQS * Wq[:, hg * HCOLS:(hg + 1) * HCOLS]
        wqh, wql = _hi_lo(np.ascontiguousarray(wq_s))
        wo8h_a, wo8l_a = _hi_lo(
            np.ascontiguousarray(WQS * Wo[hg * HCOLS:(hg + 1) * HCOLS, :])
        )
        in_maps.append(
            {
                "xh": packed_x[b][0],
                "xl": packed_x[b][1],
                "wqh": np.ascontiguousarray(
                    wqh.reshape(KP, 2, P, HCOLS).transpose(2, 0, 1, 3)
                ),
                "wql": np.ascontiguousarray(
                    wql.reshape(KP, 2, P, HCOLS).transpose(2, 0, 1, 3)
                ),
                "wo8h": np.ascontiguousarray(
                    wo8h_a.reshape(HG, P, D).transpose(1, 0, 2)
                ),
                "wo8l": np.ascontiguousarray(
                    wo8l_a.reshape(HG, P, D).transpose(1, 0, 2)
                ),
            }
        )

    res = run_bass_kernel_spmd(nc, in_maps, list(range(8)))
    _CACHE["last_res"] = res
    out = np.zeros((B, N, D), dtype=np.float32)
    for core in range(8):
        b = core // 4
        out[b] += res.results[core]["out"].astype(np.float32)
    out += bo
    return out
